# revision 9
# baseline (speedup 1.0000x reference)
"""Chamfer distance (adv->ori direction) Trainium2 Bass kernel.

Problem: adv_pc [8, 4096, 3], ori_pc [8, 4096, 3], weights [8] ->
scalar f32 loss = mean_b( w_b * mean_k( min_j ||adv_bk - ori_bj||^2 ) ).

Sharding: data parallel over the batch dim — core b handles batch b.

Per-core algorithm (K = 4096 points, d = 3):
  m[k, j]   = b2_j - 2 a_k . b_j          (augmented matmul, contract dim 4)
  out_core  = sum_k ( a2_k + min_j m[k,j] )       (= 4096 * loss1_b)
The a2_k term is added per-point BEFORE the sum over k (the min is ~ -3.0
and a2 ~ +3.0; their sum is ~0.002, so summing them separately would lose
precision to cancellation).

Matmul form: lhsT = ahatT [4, 128]   rows (ax, ay, az, 1)   per k-tile
             rhs  = bhat  [4, 4096]  rows (-2bx, -2by, -2bz, b2)
             psum[t] = lhsT.T @ rhs  -> [128, j]   then DVE min-reduce over j.

The [coord, point] layouts are built on-chip with PE transposes of
[128 points, 4] tiles (identity-matmul), scaled during the PSUM->SBUF
copy on the Scalar engine (per-partition scale (-2,-2,-2,1) for ori).
"""

import numpy as np

B = 8
K = 4096
KT = K // 128  # 32 k-tiles of 128 adv points
NCORES = 8

_NC_CACHE = {}


def _build_nc():
    import concourse.bacc as bacc
    import concourse.mybir as mybir
    import concourse.tile as tile
    from concourse import masks

    f32 = mybir.dt.float32
    Alu = mybir.AluOpType
    Act = mybir.ActivationFunctionType
    Ax = mybir.AxisListType

    nc = bacc.Bacc("TRN2", target_bir_lowering=False, debug=False,
                   num_devices=NCORES)

    adv = nc.dram_tensor("adv", [K, 3], f32, kind="ExternalInput").ap()
    ori = nc.dram_tensor("ori", [K, 3], f32, kind="ExternalInput").ap()
    out = nc.dram_tensor("out", [1, 1], f32, kind="ExternalOutput").ap()

    with tile.TileContext(nc) as tc:
        with tc.tile_pool(name="consts", bufs=1) as consts, \
             tc.tile_pool(name="sb", bufs=1) as sb:
            ident = consts.tile([128, 128], f32)
            masks.make_identity(nc, ident[:])

            # Point-major staging tiles: col 4t+d = coord d of k-tile t,
            # col 4t+3 = 1.0 (adv) / b2 (ori).
            Av = sb.tile([128, 4 * KT], f32)
            Ov = sb.tile([128, 4 * KT], f32)
            nc.gpsimd.memset(Av[:], 1.0)

            # DMA [4096, 3] -> [128 part, 32 t, 3 d] strided views (4 chunks
            # each, to spread across DMA queues).
            adv_v = adv.rearrange("(t p) d -> p t d", p=128)
            ori_v = ori.rearrange("(t p) d -> p t d", p=128)
            Av_v = Av[:].rearrange("p (t q) -> p t q", q=4)[:, :, 0:3]
            Ov_v = Ov[:].rearrange("p (t q) -> p t q", q=4)[:, :, 0:3]
            nc.sync.dma_start(out=Av_v[:], in_=adv_v[:])
            nc.sync.dma_start(out=Ov_v[:], in_=ori_v[:])

            # a2 per adv point -> a2arr [128, 32]; b2 per ori point -> col
            # 4t+3 of Ov.
            Asq = sb.tile([128, 3 * KT], f32)
            Osq = sb.tile([128, 3 * KT], f32)
            a2arr = sb.tile([128, KT], f32)
            nc.vector.tensor_tensor(Asq[:], Av_v, Av_v, op=Alu.mult)
            nc.vector.tensor_tensor(Osq[:], Ov_v, Ov_v, op=Alu.mult)
            Asq_v = Asq[:].rearrange("p (t q) -> p t q", q=3)
            Osq_v = Osq[:].rearrange("p (t q) -> p t q", q=3)
            nc.vector.tensor_reduce(a2arr[:], Asq_v, axis=Ax.X, op=Alu.add)
            b2_cols = Ov[:].rearrange("p (t q) -> p t q", q=4)[:, :, 3:4]
            nc.vector.tensor_reduce(b2_cols, Osq_v, axis=Ax.X, op=Alu.add)
            # scale ori coords in place by -2 (b2 column stays unscaled)
            nc.vector.tensor_scalar_mul(Ov_v, Ov_v, -2.0)

            # Build the [coord, point] operands with PE transposes.
            ahatT = sb.tile([4, K], f32)  # rows (ax, ay, az, 1)
            bhat = sb.tile([4, K], f32)   # rows (-2bx, -2by, -2bz, b2)
            with tc.tile_pool(name="tp", bufs=4, space="PSUM") as tp:
                for src, dst in ((Ov, bhat), (Av, ahatT)):
                    for g in range(KT // 4):
                        tpt = tp.tile([4, 512], f32, tag="tpt")
                        for i in range(4):
                            t = 4 * g + i
                            nc.tensor.transpose(
                                tpt[:, i * 128:(i + 1) * 128],
                                src[:, 4 * t:4 * t + 4],
                                ident[:],
                            )
                        nc.scalar.copy(dst[:, g * 512:(g + 1) * 512], tpt[:])

            # Main loop: per k-tile, 8 matmuls of [4,128]^T @ [4,512] into
            # PSUM, min-reduced over j in two [128, 2048] halves.
            gminP = sb.tile([128, 2 * KT], f32)
            with tc.tile_pool(name="mm", bufs=2, space="PSUM") as mm:
                for t in range(KT):
                    lhsT = ahatT[:, t * 128:(t + 1) * 128]
                    for h in range(2):
                        ps = mm.tile([128, 2048], f32, tag="ps")
                        for jb in range(4):
                            j0 = (h * 4 + jb) * 512
                            nc.tensor.matmul(
                                ps[:, jb * 512:(jb + 1) * 512],
                                lhsT, bhat[:, j0:j0 + 512],
                                start=True, stop=True,
                            )
                        c = 2 * t + h
                        nc.vector.tensor_reduce(
                            gminP[:, c:c + 1], ps[:], axis=Ax.X, op=Alu.min)

                # Combine: min over the two halves, add a2 per point, sum.
                gmin2 = sb.tile([128, KT], f32)
                tot = sb.tile([128, KT], f32)
                ksum = sb.tile([128, 1], f32)
                res = sb.tile([1, 1], f32)
                gminP_v = gminP[:].rearrange("p (t h) -> p t h", h=2)
                nc.vector.tensor_reduce(gmin2[:], gminP_v, axis=Ax.X,
                                        op=Alu.min)
                nc.vector.tensor_tensor(tot[:], gmin2[:], a2arr[:],
                                        op=Alu.add)
                nc.vector.tensor_reduce(ksum[:], tot[:], axis=Ax.X,
                                        op=Alu.add)
                ps = mm.tile([128, 2048], f32, tag="ps")
                nc.tensor.matmul(ps[:1, :1], ksum[:], Av[:, 3:4],
                                 start=True, stop=True)
                nc.scalar.copy(res[:], ps[:1, :1])
                nc.sync.dma_start(out=out[:], in_=res[:])

    nc.compile()
    return nc


def _get_nc():
    if "nc" not in _NC_CACHE:
        _NC_CACHE["nc"] = _build_nc()
    return _NC_CACHE["nc"]


def kernel(adv_pc, ori_pc, weights):
    from concourse.bass_utils import run_bass_kernel_spmd

    adv_pc = np.asarray(adv_pc, dtype=np.float32)
    ori_pc = np.asarray(ori_pc, dtype=np.float32)
    weights = np.asarray(weights, dtype=np.float32)

    nc = _get_nc()
    in_maps = [
        {"adv": np.ascontiguousarray(adv_pc[b]),
         "ori": np.ascontiguousarray(ori_pc[b])}
        for b in range(B)
    ]
    res = run_bass_kernel_spmd(nc, in_maps, core_ids=list(range(NCORES)))
    sums = np.array([res.results[b]["out"][0, 0] for b in range(B)],
                    dtype=np.float32)
    loss1 = sums / np.float32(K)
    return np.array(np.mean(loss1 * weights), dtype=np.float32)


if __name__ == "__main__":
    rng = np.random.default_rng(0)
    a = rng.standard_normal((B, K, 3), dtype=np.float32)
    o = rng.standard_normal((B, K, 3), dtype=np.float32)
    w = np.ones((B,), dtype=np.float32)
    print(kernel(a, o, w))


# revision 11
# speedup vs baseline: 2.4484x; 2.4484x over previous
"""Chamfer distance (adv->ori direction) Trainium2 Bass kernel.

Problem: adv_pc [8, 4096, 3], ori_pc [8, 4096, 3], weights [8] ->
scalar f32 loss = mean_b( w_b * mean_k( min_j ||adv_bk - ori_bj||^2 ) ).

Sharding: data parallel over the batch dim — core b handles batch b.

Per-core algorithm (K = 4096 points, d = 3):
  m[k, j]   = b2_j - 2 a_k . b_j          (augmented matmul, contract dim 4)
  out_core  = sum_k ( a2_k + min_j m[k,j] )       (= 4096 * loss1_b)
The a2_k term is added per-point BEFORE the sum over k (the min is ~ -3.0
and a2 ~ +3.0; their sum is ~0.002, so summing them separately would lose
precision to cancellation).

Matmul form: lhsT = ahatT [4, 128]   rows (ax, ay, az, 1)   per k-tile
             rhs  = bhat  [4, 4096]  rows (-2bx, -2by, -2bz, b2)
             psum[t] = lhsT.T @ rhs  -> [128, j]   then DVE min-reduce over j.

The [coord, point] layouts are built on-chip with PE transposes of
[128 points, 4] tiles (identity-matmul), scaled during the PSUM->SBUF
copy on the Scalar engine (per-partition scale (-2,-2,-2,1) for ori).
"""

import numpy as np

B = 8
K = 4096
KT = K // 128  # 32 k-tiles of 128 adv points
NCORES = 8

_NC_CACHE = {}


def _build_nc():
    import concourse.bacc as bacc
    import concourse.mybir as mybir
    import concourse.tile as tile
    from concourse import masks

    f32 = mybir.dt.float32
    Alu = mybir.AluOpType
    Act = mybir.ActivationFunctionType
    Ax = mybir.AxisListType

    nc = bacc.Bacc("TRN2", target_bir_lowering=False, debug=False,
                   num_devices=NCORES)

    adv = nc.dram_tensor("adv", [K, 3], f32, kind="ExternalInput").ap()
    ori = nc.dram_tensor("ori", [K, 3], f32, kind="ExternalInput").ap()
    out = nc.dram_tensor("out", [1, 1], f32, kind="ExternalOutput").ap()

    with tile.TileContext(nc) as tc:
        with tc.tile_pool(name="consts", bufs=1) as consts, \
             tc.tile_pool(name="sb", bufs=1) as sb:
            ident = consts.tile([128, 128], f32)
            masks.make_identity(nc, ident[:])

            # Point-major staging tiles: col 4t+d = coord d of k-tile t,
            # col 4t+3 = 1.0 (adv) / b2 (ori).
            Av = sb.tile([128, 4 * KT], f32)
            Ov = sb.tile([128, 4 * KT], f32)
            nc.gpsimd.memset(Av[:], 1.0)

            # DMA [4096, 3] -> [128 part, 32 t, 3 d] strided views (4 chunks
            # each, to spread across DMA queues).
            adv_v = adv.rearrange("(t p) d -> p t d", p=128)
            ori_v = ori.rearrange("(t p) d -> p t d", p=128)
            Av_v = Av[:].rearrange("p (t q) -> p t q", q=4)[:, :, 0:3]
            Ov_v = Ov[:].rearrange("p (t q) -> p t q", q=4)[:, :, 0:3]
            nc.sync.dma_start(out=Av_v[:], in_=adv_v[:])
            nc.sync.dma_start(out=Ov_v[:], in_=ori_v[:])

            # a2 per adv point -> a2arr [128, 32]; b2 per ori point -> col
            # 4t+3 of Ov.
            Asq = sb.tile([128, 3 * KT], f32)
            Osq = sb.tile([128, 3 * KT], f32)
            a2arr = sb.tile([128, KT], f32)
            nc.vector.tensor_tensor(Asq[:], Av_v, Av_v, op=Alu.mult)
            nc.vector.tensor_tensor(Osq[:], Ov_v, Ov_v, op=Alu.mult)
            Asq_v = Asq[:].rearrange("p (t q) -> p t q", q=3)
            Osq_v = Osq[:].rearrange("p (t q) -> p t q", q=3)
            nc.vector.tensor_reduce(a2arr[:], Asq_v, axis=Ax.X, op=Alu.add)
            b2_cols = Ov[:].rearrange("p (t q) -> p t q", q=4)[:, :, 3:4]
            nc.vector.tensor_reduce(b2_cols, Osq_v, axis=Ax.X, op=Alu.add)
            # scale ori coords in place by -2 (b2 column stays unscaled)
            nc.vector.tensor_scalar_mul(Ov_v, Ov_v, -2.0)

            # Build the [coord, point] operands with PE transposes, then
            # replicate rows 0..3 into PE row groups 32/64/96 (SBUF->SBUF
            # DMA) so four matmuls can run concurrently via tile_position.
            ahatT = sb.tile([128, K], f32)  # rows 32g+(0..3) = (ax, ay, az, 1)
            bhat = sb.tile([128, K], f32)   # rows 32g+(0..3) = (-2bx,..., b2)
            with tc.tile_pool(name="tp", bufs=4, space="PSUM") as tp:
                for src, dst in ((Ov, bhat), (Av, ahatT)):
                    for g in range(KT // 4):
                        tpt = tp.tile([4, 512], f32, tag="tpt")
                        for i in range(4):
                            t = 4 * g + i
                            nc.tensor.transpose(
                                tpt[:, i * 128:(i + 1) * 128],
                                src[:, 4 * t:4 * t + 4],
                                ident[:],
                            )
                        blk = dst[0:4, g * 512:(g + 1) * 512]
                        nc.scalar.copy(blk, tpt[:])
                        for r in (32, 64, 96):
                            nc.sync.dma_start(
                                out=dst[r:r + 4, g * 512:(g + 1) * 512],
                                in_=blk)

            # Main loop: per k-tile, 8 matmuls of [4,128]^T @ [4,512] into
            # PSUM, min-reduced over j in two [128, 2048] halves.
            gminP = sb.tile([128, 2 * KT], f32)
            with tc.tile_pool(name="mm", bufs=2, space="PSUM") as mm:
                for t in range(KT):
                    for h in range(2):
                        ps = mm.tile([128, 2048], f32, tag="ps")
                        for g in range(4):
                            j0 = (h * 4 + g) * 512
                            r = 32 * g
                            nc.tensor.matmul(
                                ps[:, g * 512:(g + 1) * 512],
                                ahatT[r:r + 4, t * 128:(t + 1) * 128],
                                bhat[r:r + 4, j0:j0 + 512],
                                start=True, stop=True,
                                tile_position=(r, 0),
                            )
                        c = 2 * t + h
                        nc.vector.tensor_reduce(
                            gminP[:, c:c + 1], ps[:], axis=Ax.X, op=Alu.min)

                # Combine: min over the two halves, add a2 per point, sum.
                gmin2 = sb.tile([128, KT], f32)
                tot = sb.tile([128, KT], f32)
                ksum = sb.tile([128, 1], f32)
                res = sb.tile([1, 1], f32)
                gminP_v = gminP[:].rearrange("p (t h) -> p t h", h=2)
                nc.vector.tensor_reduce(gmin2[:], gminP_v, axis=Ax.X,
                                        op=Alu.min)
                nc.vector.tensor_tensor(tot[:], gmin2[:], a2arr[:],
                                        op=Alu.add)
                nc.vector.tensor_reduce(ksum[:], tot[:], axis=Ax.X,
                                        op=Alu.add)
                ps = mm.tile([128, 2048], f32, tag="ps")
                nc.tensor.matmul(ps[:1, :1], ksum[:], Av[:, 3:4],
                                 start=True, stop=True)
                nc.scalar.copy(res[:], ps[:1, :1])
                nc.sync.dma_start(out=out[:], in_=res[:])

    nc.compile()
    return nc


def _get_nc():
    if "nc" not in _NC_CACHE:
        _NC_CACHE["nc"] = _build_nc()
    return _NC_CACHE["nc"]


def kernel(adv_pc, ori_pc, weights):
    from concourse.bass_utils import run_bass_kernel_spmd

    adv_pc = np.asarray(adv_pc, dtype=np.float32)
    ori_pc = np.asarray(ori_pc, dtype=np.float32)
    weights = np.asarray(weights, dtype=np.float32)

    nc = _get_nc()
    in_maps = [
        {"adv": np.ascontiguousarray(adv_pc[b]),
         "ori": np.ascontiguousarray(ori_pc[b])}
        for b in range(B)
    ]
    res = run_bass_kernel_spmd(nc, in_maps, core_ids=list(range(NCORES)))
    sums = np.array([res.results[b]["out"][0, 0] for b in range(B)],
                    dtype=np.float32)
    loss1 = sums / np.float32(K)
    return np.array(np.mean(loss1 * weights), dtype=np.float32)


if __name__ == "__main__":
    rng = np.random.default_rng(0)
    a = rng.standard_normal((B, K, 3), dtype=np.float32)
    o = rng.standard_normal((B, K, 3), dtype=np.float32)
    w = np.ones((B,), dtype=np.float32)
    print(kernel(a, o, w))
